# revision 1
# baseline (speedup 1.0000x reference)
"""Differential self-attention on 8 Trainium2 NeuronCores.

Sharding: batch x head-group. Core c handles batch b = c//4 and heads
hs = 4*(c%4) .. 4*(c%4)+4 (4 of 16 heads). Each core computes q/k/v
projections for its heads, RoPE, both causal softmax score matrices
(flash-style, unnormalized, with matmul-computed row sums), the
differential combination + RMS norm, and a partial out-projection over
its heads' dims. Host sums the 4 partial y per batch and adds bo.

Per-core kernel layouts (feature-major "T" = [feat, seq]):
  qT/kT  [128, 4, 2048]  rows 0:64 = component 1, 64:128 = component 2
  v      [128, 16, 4, 65] (key-block, head, hd + ones col [unused])
  scores sT [j=128, i<=512] so attn@v contracts j on partitions.
"""

import math

import numpy as np
import ml_dtypes

B, S, D = 2, 2048, 1024
H, HD = 16, 64
HALF = HD // 2
NCORES = 8
NH = 4            # heads per core
LAMBDA_INIT = 0.2
EPS = 1e-6
CH = 512          # query chunk
NCH = S // CH
JB = 128          # key block
BF16 = ml_dtypes.bfloat16

_cache = {}


def _build():
    import concourse.bass as bass
    import concourse.tile as tile
    from concourse import bacc, mybir

    F32R = mybir.dt.float32r
    F32 = mybir.dt.float32
    BF = mybir.dt.bfloat16
    AF = mybir.ActivationFunctionType

    nc = bacc.Bacc("TRN2", debug=False, num_devices=NCORES)

    xT = nc.dram_tensor("xT", [D, S], F32R, kind="ExternalInput")
    wqT = nc.dram_tensor("wqT", [128, 8, 512], F32R, kind="ExternalInput")
    wkT = nc.dram_tensor("wkT", [128, 8, 512], F32R, kind="ExternalInput")
    wvT = nc.dram_tensor("wvT", [128, 8, 256], F32R, kind="ExternalInput")
    bqk = nc.dram_tensor("bqk", [8, 128], F32R, kind="ExternalInput")
    bv = nc.dram_tensor("bv", [1, 256], F32R, kind="ExternalInput")
    woT = nc.dram_tensor("woT", [128, 2, 1024], F32R, kind="ExternalInput")
    ropeA = nc.dram_tensor("ropeA", [128, S], BF, kind="ExternalInput")
    ropeB = nc.dram_tensor("ropeB", [128, S], BF, kind="ExternalInput")
    maskd = nc.dram_tensor("maskd", [128, 128], BF, kind="ExternalInput")
    lamv = nc.dram_tensor("lamv", [128, 1], F32, kind="ExternalInput")
    iim = nc.dram_tensor("iim", [128, 64], BF, kind="ExternalInput")
    onesd = nc.dram_tensor("onesd", [1, 512], F32R, kind="ExternalInput")
    ones128d = nc.dram_tensor("ones128d", [128, 2], F32R, kind="ExternalInput")
    y_out = nc.dram_tensor("y", [S, D], F32, kind="ExternalOutput")

    xT_r = xT.ap().rearrange("(t p) s -> p t s", p=128)

    with tile.TileContext(nc) as tc:
        import contextlib
        ctx = contextlib.ExitStack()
        with ctx:
            persist = ctx.enter_context(tc.tile_pool(name="persist", bufs=1))
            xpool = ctx.enter_context(tc.tile_pool(name="xc", bufs=2))
            rpool = ctx.enter_context(tc.tile_pool(name="rope", bufs=2))
            apool = ctx.enter_context(tc.tile_pool(name="atile", bufs=4))
            npool = ctx.enter_context(tc.tile_pool(name="norm", bufs=2))
            ofpool = ctx.enter_context(tc.tile_pool(name="of", bufs=2))
            ypool = ctx.enter_context(tc.tile_pool(name="y", bufs=2))
            psum = ctx.enter_context(tc.tile_pool(name="ps", bufs=2, space="PSUM"))
            dpool = ctx.enter_context(tc.tile_pool(name="dscr", bufs=4, space="DRAM"))

            # ---- persistent tiles
            wq_sb = persist.tile([128, 8, 512], F32R, tag="wq")
            nc.sync.dma_start(out=wq_sb[:], in_=wqT.ap())
            wk_sb = persist.tile([128, 8, 512], F32R, tag="wk")
            nc.sync.dma_start(out=wk_sb[:], in_=wkT.ap())
            wv_sb = persist.tile([128, 8, 256], F32R, tag="wv")
            nc.sync.dma_start(out=wv_sb[:], in_=wvT.ap())
            wo_sb = persist.tile([128, 2, 1024], F32R, tag="wo")
            nc.sync.dma_start(out=wo_sb[:], in_=woT.ap())
            bqk_sb = persist.tile([1, 8, 128], F32R, tag="bqk")
            nc.sync.dma_start(out=bqk_sb[:], in_=bqk.ap())
            bv_sb = persist.tile([1, 256], F32R, tag="bv")
            nc.sync.dma_start(out=bv_sb[:], in_=bv.ap())
            rA_sb = persist.tile([128, S], BF, tag="ra")
            nc.sync.dma_start(out=rA_sb[:], in_=ropeA.ap())
            rB_sb = persist.tile([128, S], BF, tag="rb")
            nc.sync.dma_start(out=rB_sb[:], in_=ropeB.ap())
            mask_sb = persist.tile([128, 128], BF, tag="mask")
            nc.sync.dma_start(out=mask_sb[:], in_=maskd.ap())
            lam_sb = persist.tile([128, 1], F32, tag="lam")
            nc.sync.dma_start(out=lam_sb[:], in_=lamv.ap())
            ii_sb = persist.tile([128, 64], BF, tag="ii")
            nc.sync.dma_start(out=ii_sb[:], in_=iim.ap())

            qT_sb = persist.tile([128, NH, S], BF, tag="qT")
            kT_sb = persist.tile([128, NH, S], BF, tag="kT")
            v_sb = persist.tile([128, 16, NH, 64], BF, tag="v")

            onesr = persist.tile([1, 512], F32R, tag="onesr")
            nc.sync.dma_start(out=onesr[:], in_=onesd.ap())
            ones128 = persist.tile([128, 2], BF, tag="ones128")
            nc.vector.memset(ones128[:], 1.0)
            ones128r = persist.tile([128, 2], F32R, tag="ones128r")
            nc.sync.dma_start(out=ones128r[:], in_=ones128d.ap())
            epst = persist.tile([128, 1], F32, tag="epst")
            nc.vector.memset(epst[:], EPS)

            swap_src = [32, 0, 96, 64]

            for c in range(NCH):
                cs = c * CH
                # ======== projections for this chunk ========
                xc = xpool.tile([128, 8, CH], F32R)
                nc.sync.dma_start(out=xc[:], in_=xT_r[:, :, cs:cs + CH])

                # q/k feature tiles (f 0..3 = q heads, 4..7 = k heads)
                for f in range(8):
                    ps = psum.tile([128, 512], F32, tag="o")
                    wsb = wq_sb if f < 4 else wk_sb
                    fi = f % 4
                    for kt in range(8):
                        nc.tensor.matmul(
                            ps[:], lhsT=wsb[:, kt, fi * 128:fi * 128 + 128],
                            rhs=xc[:, kt, :], start=(kt == 0), stop=False)
                    nc.tensor.matmul(
                        ps[:], lhsT=bqk_sb[:, f, :], rhs=onesr[:],
                        start=False, stop=True)
                    qc = rpool.tile([128, CH], BF, tag="qc")
                    nc.scalar.copy(qc[:], ps[:])
                    qs = rpool.tile([128, CH], BF, tag="qs")
                    for g in range(4):
                        nc.sync.dma_start(
                            out=qs[g * 32:g * 32 + 32, :],
                            in_=qc[swap_src[g]:swap_src[g] + 32, :])
                    t1 = rpool.tile([128, CH], BF, tag="t1")
                    nc.vector.tensor_mul(t1[:], qc[:], rA_sb[:, cs:cs + CH])
                    t2 = rpool.tile([128, CH], BF, tag="t2")
                    nc.vector.tensor_mul(t2[:], qs[:], rB_sb[:, cs:cs + CH])
                    dst = (qT_sb if f < 4 else kT_sb)[:, fi, cs:cs + CH]
                    nc.vector.tensor_add(dst, t1[:], t2[:])

                # v for the 4 key blocks of this chunk (seq-major)
                for m in range(4):
                    ps = psum.tile([128, 512], F32, tag="o")
                    for kt in range(8):
                        nc.tensor.matmul(
                            ps[:, 0:256], lhsT=xc[:, kt, m * 128:m * 128 + 128],
                            rhs=wv_sb[:, kt, :], start=(kt == 0), stop=False)
                    nc.tensor.matmul(
                        ps[:, 0:256], lhsT=onesr[:, 0:128], rhs=bv_sb[:],
                        start=False, stop=True)
                    nc.vector.tensor_copy(
                        v_sb[:, 4 * c + m, :, :],
                        ps[:, 0:256].rearrange("p (h d) -> p h d", h=4))

                # ======== attention for this chunk ========
                njb = 4 * c + 4
                sums1 = psum.tile([128, 512], F32, tag="sums")
                sums2 = psum.tile([128, 512], F32, tag="sums")
                r1t = npool.tile([128, 512], F32, tag="r1t")
                r2t = npool.tile([128, 512], F32, tag="r2t")
                of_sb = ofpool.tile([128, 2, CH], F32R, tag="of")

                for hg in range(2):
                    o_ps = [psum.tile([128, 512], F32, tag="o",
                                      name=f"o_ps_{c}_{hg}_{i}")
                            for i in range(2)]
                    for jb in range(njb):
                        i0 = max(0, (jb - 4 * c) * 128)
                        for hh in range(2):
                            h = 2 * hg + hh
                            sc = psum.tile([128, 2, 512], F32, tag="score")
                            nc.tensor.matmul(
                                sc[:, 0, i0:512],
                                lhsT=kT_sb[0:64, h, jb * JB:jb * JB + JB],
                                rhs=qT_sb[0:64, h, cs + i0:cs + CH],
                                start=True, stop=True)
                            nc.tensor.matmul(
                                sc[:, 1, i0:512],
                                lhsT=kT_sb[64:128, h, jb * JB:jb * JB + JB],
                                rhs=qT_sb[64:128, h, cs + i0:cs + CH],
                                start=True, stop=True)
                            at = apool.tile([128, 2, 512], BF, tag="at")
                            nc.scalar.activation(
                                at[:, :, i0:512], sc[:, :, i0:512], AF.Exp,
                                scale=0.125)
                            if jb >= 4 * c:
                                nc.vector.tensor_mul(
                                    at[:, 0, i0:i0 + 128],
                                    at[:, 0, i0:i0 + 128], mask_sb[:])
                                nc.vector.tensor_mul(
                                    at[:, 1, i0:i0 + 128],
                                    at[:, 1, i0:i0 + 128], mask_sb[:])
                            vsl = v_sb[:, jb, h, :]
                            nc.tensor.matmul(
                                o_ps[hh][0:64, i0:512], lhsT=vsl,
                                rhs=at[:, 0, i0:512], start=(jb == 0),
                                stop=(jb == njb - 1), tile_position=(0, 0))
                            nc.tensor.matmul(
                                o_ps[hh][64:128, i0:512], lhsT=vsl,
                                rhs=at[:, 1, i0:512], start=(jb == 0),
                                stop=(jb == njb - 1), tile_position=(0, 64))
                            nc.tensor.matmul(
                                sums1[32 * h:32 * h + 1, i0:512],
                                lhsT=ones128[:, 0:1], rhs=at[:, 0, i0:512],
                                start=(jb == 0), stop=(jb == njb - 1),
                                tile_position=(0, 32 * h))
                            nc.tensor.matmul(
                                sums2[32 * h:32 * h + 1, i0:512],
                                lhsT=ones128[:, 0:1], rhs=at[:, 1, i0:512],
                                start=(jb == 0), stop=(jb == njb - 1),
                                tile_position=(0, 32 * h))

                    # ---- normalize + combine the two heads of this group
                    p0 = 64 * hg
                    nc.vector.tensor_copy(r1t[p0:p0 + 33, :],
                                          sums1[p0:p0 + 33, :])
                    nc.vector.tensor_copy(r2t[p0:p0 + 33, :],
                                          sums2[p0:p0 + 33, :])
                    nc.scalar.activation(r1t[p0:p0 + 33, :],
                                         r1t[p0:p0 + 33, :], AF.Ln)
                    nc.scalar.activation(r1t[p0:p0 + 33, :],
                                         r1t[p0:p0 + 33, :], AF.Exp,
                                         scale=-1.0)
                    nc.scalar.activation(r2t[p0:p0 + 33, :],
                                         r2t[p0:p0 + 33, :], AF.Ln)
                    nc.scalar.activation(r2t[p0:p0 + 33, :],
                                         r2t[p0:p0 + 33, :], AF.Exp,
                                         scale=-1.0)
                    nc.vector.tensor_scalar(
                        r2t[p0:p0 + 33, :], r2t[p0:p0 + 33, :],
                        lam_sb[p0:p0 + 33, 0:1], None,
                        op0=mybir.AluOpType.mult)

                    occs = []
                    inv_bf = npool.tile([128, 512], F32, tag="invbf")
                    sqs = npool.tile([128, 512], F32, tag="sqs")
                    ssq = psum.tile([128, 512], F32, tag="score")
                    for hh in range(2):
                        h = 2 * hg + hh
                        row = p0 + 32 * hh
                        rb = npool.tile([128, 512], F32, tag="rbb")
                        d1 = dpool.tile([1, 512], F32, tag="d1",
                                        name=f"d1_{c}_{h}")
                        nc.sync.dma_start(out=d1[:], in_=r1t[row:row + 1, :])
                        nc.sync.dma_start(
                            out=rb[0:64, :], in_=_bcast_dram(bass, d1, 64))
                        d2 = dpool.tile([1, 512], F32, tag="d2",
                                        name=f"d2_{c}_{h}")
                        nc.sync.dma_start(out=d2[:], in_=r2t[row:row + 1, :])
                        nc.sync.dma_start(
                            out=rb[64:128, :], in_=_bcast_dram(bass, d2, 64))
                        tstack = npool.tile([128, 512], BF, tag="tstack")
                        nc.vector.tensor_mul(tstack[0:64, :],
                                             o_ps[hh][0:64, :], rb[0:64, :])
                        nc.vector.tensor_mul(tstack[64:128, :],
                                             o_ps[hh][64:128, :],
                                             rb[64:128, :])
                        oc = psum.tile([128, 512], F32, tag="score")
                        nc.tensor.matmul(
                            oc[64 * hh:64 * hh + 64, :], lhsT=ii_sb[:],
                            rhs=tstack[:], start=True, stop=True,
                            tile_position=(0, 64 * hh))
                        occ = npool.tile([128, 512], F32, tag="occ")
                        nc.vector.tensor_copy(occ[64 * hh:64 * hh + 64, :],
                                              oc[64 * hh:64 * hh + 64, :])
                        sq = npool.tile([128, 512], BF, tag="sq")
                        nc.vector.tensor_mul(sq[64 * hh:64 * hh + 64, :],
                                             occ[64 * hh:64 * hh + 64, :],
                                             occ[64 * hh:64 * hh + 64, :])
                        nc.tensor.matmul(
                            ssq[32 * h:32 * h + 1, :],
                            lhsT=ones128[64 * hh:64 * hh + 64, 0:1],
                            rhs=sq[64 * hh:64 * hh + 64, :],
                            start=True, stop=True,
                            tile_position=(64 * hh, 32 * h))
                        occs.append(occ)
                        if hh == 1:
                            nc.vector.tensor_copy(sqs[p0:p0 + 33, :],
                                                  ssq[p0:p0 + 33, :])
                            nc.scalar.activation(sqs[p0:p0 + 33, :],
                                                 sqs[p0:p0 + 33, :], AF.Ln,
                                                 scale=1.0 / 64.0,
                                                 bias=epst[p0:p0 + 33, 0:1])
                            nc.scalar.activation(inv_bf[p0:p0 + 33, :],
                                                 sqs[p0:p0 + 33, :], AF.Exp,
                                                 scale=-0.5)
                    for hh in range(2):
                        h = 2 * hg + hh
                        row = p0 + 32 * hh
                        invb = npool.tile([128, 512], F32, tag="invb")
                        d3 = dpool.tile([1, 512], F32, tag="d3",
                                        name=f"d3_{c}_{h}")
                        nc.sync.dma_start(out=d3[:],
                                          in_=inv_bf[row:row + 1, :])
                        nc.sync.dma_start(
                            out=invb[64 * hh:64 * hh + 64, :],
                            in_=_bcast_dram(bass, d3, 64))
                        occ = occs[hh]
                        nc.vector.tensor_mul(
                            of_sb[64 * hh:64 * hh + 64, hg, :],
                            occ[64 * hh:64 * hh + 64, :],
                            invb[64 * hh:64 * hh + 64, :])

                # ======== partial out-projection for this chunk ========
                for t in range(4):
                    for n in range(2):
                        yp = psum.tile([128, 512], F32, tag="o")
                        for kt in range(2):
                            nc.tensor.matmul(
                                yp[:], lhsT=of_sb[:, kt, t * 128:t * 128 + 128],
                                rhs=wo_sb[:, kt, n * 512:n * 512 + 512],
                                start=(kt == 0), stop=(kt == 1))
                        ys = ypool.tile([128, 512], F32, tag="ys")
                        nc.vector.tensor_copy(ys[:], yp[:])
                        nc.sync.dma_start(
                            out=y_out.ap()[cs + t * 128:cs + t * 128 + 128,
                                           n * 512:n * 512 + 512],
                            in_=ys[:])

    nc.compile()
    return nc


def _bcast_dram(bass_mod, dtile, nparts):
    """AP reading a [1, N] DRAM scratch tile nparts times (row broadcast)."""
    ap = dtile[:]
    return bass_mod.AP(tensor=ap.tensor, offset=ap.offset,
                       ap=[[0, nparts]] + ap.ap[1:])


def _prep_inputs(x, Wq, bq, Wk, bk, Wv, bv, Wo, bo, head_norm_w,
                 lq1, lk1, lq2, lk2):
    lam_full = (LAMBDA_INIT
                + np.exp(np.sum(lq1 * lk1, -1))
                - np.exp(np.sum(lq2 * lk2, -1)))  # [H]

    half = HALF
    inv_freq = 1.0 / (10000.0 ** (np.arange(half, dtype=np.float64) / half))
    ang = np.arange(S, dtype=np.float64)[:, None] * inv_freq[None, :]  # [S,32]
    cosT = np.cos(ang).T.astype(np.float32)  # [32, S]
    sinT = np.sin(ang).T.astype(np.float32)
    ropeA = np.tile(cosT, (4, 1)).astype(BF16)                      # [128,S]
    ropeB = np.concatenate([-sinT, sinT, -sinT, sinT], 0).astype(BF16)

    maskd = np.triu(np.ones((128, 128), np.float32)).astype(BF16)   # j<=i
    iim = np.zeros((128, 64), np.float32)
    iim[np.arange(128), np.arange(128) % 64] = 1.0
    iim = iim.astype(BF16)

    in_maps = []
    for c in range(NCORES):
        b = c // 4
        h0 = 4 * (c % 4)
        rq = slice(h0 * 128, h0 * 128 + 512)
        rv = slice(h0 * 64, h0 * 64 + 256)

        xTc = np.ascontiguousarray(x[b].T)                          # [D, S]
        wq_l = Wq[rq].T  # [1024, 512]
        wk_l = Wk[rq].T
        wv_l = Wv[rv].T  # [1024, 256]
        wqr = np.ascontiguousarray(
            wq_l.reshape(8, 128, 512).transpose(1, 0, 2))
        wkr = np.ascontiguousarray(
            wk_l.reshape(8, 128, 512).transpose(1, 0, 2))
        wvr = np.ascontiguousarray(
            wv_l.reshape(8, 128, 256).transpose(1, 0, 2))

        hnw = head_norm_w[h0:h0 + 4].reshape(256)                   # local dims
        wo_l = Wo[:, rv].T * (hnw * (1.0 - LAMBDA_INIT))[:, None]   # [256,1024]
        wor = np.ascontiguousarray(
            wo_l.reshape(2, 128, 1024).transpose(1, 0, 2)).astype(np.float32)

        bqk_arr = np.stack([bq[rq][f * 128:f * 128 + 128] if f < 4
                            else bk[rq][(f - 4) * 128:(f - 4) * 128 + 128]
                            for f in range(8)]).astype(np.float32)  # [8,128]
        bv_arr = bv[rv].reshape(1, 256).astype(np.float32)

        lamv = np.ones((128, 1), np.float32)
        for hl in range(4):
            lamv[32 * hl, 0] = -lam_full[h0 + hl]

        in_maps.append({
            "xT": xTc.astype(np.float32),
            "wqT": wqr.astype(np.float32),
            "wkT": wkr.astype(np.float32),
            "wvT": wvr.astype(np.float32),
            "bqk": bqk_arr,
            "bv": bv_arr,
            "woT": wor,
            "ropeA": ropeA,
            "ropeB": ropeB,
            "maskd": maskd,
            "lamv": lamv,
            "iim": iim,
            "onesd": np.ones((1, 512), np.float32),
            "ones128d": np.ones((128, 2), np.float32),
        })
    return in_maps


def kernel(**inputs):
    from concourse.bass_utils import run_bass_kernel_spmd

    if "nc" not in _cache:
        _cache["nc"] = _build()
    nc = _cache["nc"]

    inputs = {k: np.asarray(v) for k, v in inputs.items()}
    in_maps = _prep_inputs(**inputs)
    res = run_bass_kernel_spmd(nc, in_maps, list(range(NCORES)))

    bo = inputs["bo"]
    y = np.zeros((B, S, D), np.float32)
    for b in range(B):
        acc = np.zeros((S, D), np.float32)
        for c in range(4 * b, 4 * b + 4):
            acc += res.results[c]["y"]
        y[b] = acc + bo[None, :]
    return y



# revision 37
# speedup vs baseline: 1.9168x; 1.9168x over previous
"""Differential self-attention on 8 Trainium2 NeuronCores (v2).

Sharding: batch x head-group. Core c handles batch b = c//4 and heads
h0 = 4*(c%4) .. h0+4. Per core: q/k/v projections (fp8 DoubleRow
matmuls, weights pre-scaled x64 on host), RoPE (feature pairs
interleaved so rotate-half is a DVE stream_shuffle), causal scores per
128-key block (bf16, mask added as a -1e7 upper-triangular matmul into
the score psum), exp on the Act engine, position-major attn@v
(out = [queries, dims], so softmax denominators and RMS scales are
per-partition scalars), differential combine + RMS norm on DVE, PE
transpose, partial out-projection (head_norm and (1-lambda_init)
folded into Wo). Host sums the 4 partial y per batch and adds bo.
"""

import math

import numpy as np
import ml_dtypes

B, S, D = 2, 2048, 1024
H, HD = 16, 64
NCORES = 8
NH = 4            # heads per core
LAMBDA_INIT = 0.2
EPS = 1e-6
CH = 512          # query chunk
NCH = S // CH
QS = 128          # query sub-block
WS = 1.0          # weight pre-scale (1.0 for f16)
BF16 = ml_dtypes.bfloat16
F16 = np.float16
MASKV = -1.0e7

_cache = {}


def _build():
    import contextlib

    import concourse.bass as bass
    import concourse.tile as tile
    from concourse import bacc, mybir

    F32 = mybir.dt.float32
    BF = mybir.dt.bfloat16
    F16 = mybir.dt.float16
    AF = mybir.ActivationFunctionType
    MUL = mybir.AluOpType.mult
    ADD = mybir.AluOpType.add
    DIV = mybir.AluOpType.divide

    nc = bacc.Bacc("TRN2", debug=False, num_devices=NCORES)

    xdr_d = nc.dram_tensor("xdr", [128, 8, S], F16, kind="ExternalInput")
    wq_d = nc.dram_tensor("wq", [128, 4, 8, 128], F16, kind="ExternalInput")
    wk_d = nc.dram_tensor("wk", [128, 4, 8, 128], F16, kind="ExternalInput")
    wv_d = nc.dram_tensor("wv", [128, 8, 256], F16, kind="ExternalInput")
    wo_d = nc.dram_tensor("wo", [128, 2, 1024], BF, kind="ExternalInput")
    bqk_d = nc.dram_tensor("bqk", [128, 8], F32, kind="ExternalInput")
    bv_d = nc.dram_tensor("bv", [1, 256], BF, kind="ExternalInput")
    ra_d = nc.dram_tensor("ra", [128, S], BF, kind="ExternalInput")
    rb_d = nc.dram_tensor("rb", [128, S], BF, kind="ExternalInput")
    mask01_d = nc.dram_tensor("mask01", [128, 128], BF, kind="ExternalInput")
    nlam_d = nc.dram_tensor("nlam", [128, 4], F32, kind="ExternalInput")
    epsb_d = nc.dram_tensor("epsb", [128, 1], F32, kind="ExternalInput")
    ones1_d = nc.dram_tensor("ones1", [1, 128], BF, kind="ExternalInput")
    y_d = nc.dram_tensor("y", [S, D], BF, kind="ExternalOutput")

    ESCALE = 0.125 / (WS * WS)  # 1/sqrt(hd) folded with fp8 weight scale

    def bcast_free(ap, n):
        """Broadcast a [...]-shaped AP along a new innermost free dim."""
        return bass.AP(tensor=ap.tensor, offset=ap.offset, ap=ap.ap + [[0, n]])

    def bcast_mid(ap, n):
        """Broadcast a [p, m] AP to [p, n, m] (stride-0 middle dim)."""
        return bass.AP(tensor=ap.tensor, offset=ap.offset,
                       ap=[ap.ap[0], [0, n]] + ap.ap[1:])

    with tile.TileContext(nc) as tc:
        ctx = contextlib.ExitStack()
        with ctx:
            pp = ctx.enter_context(tc.tile_pool(name="pers", bufs=1))
            wk_pool = ctx.enter_context(tc.tile_pool(name="wkp", bufs=2))
            ps = ctx.enter_context(tc.tile_pool(name="ps", bufs=1, space="PSUM"))

            # ---- persistent SBUF (DMA order tuned so chunk-3 pass-0
            # projections can start as early as possible)
            wq = pp.tile([128, 4, 8, 128], F16, tag="wq")
            nc.sync.dma_start(out=wq[:, 0:2, :, :], in_=wq_d.ap()[:, 0:2, :, :])
            wk = pp.tile([128, 4, 8, 128], F16, tag="wk")
            nc.sync.dma_start(out=wk[:, 0:2, :, :], in_=wk_d.ap()[:, 0:2, :, :])
            bqk = pp.tile([128, 8], F32, tag="bqk")
            nc.sync.dma_start(out=bqk[:], in_=bqk_d.ap())
            xdr = pp.tile([128, 8, S], F16, tag="xdr")
            for g in range(8):
                nc.sync.dma_start(out=xdr[:, g, :], in_=xdr_d.ap()[:, g, :])
            ra = pp.tile([128, S], BF, tag="ra")
            nc.sync.dma_start(out=ra[:], in_=ra_d.ap())
            rb = pp.tile([128, S], BF, tag="rb")
            nc.sync.dma_start(out=rb[:], in_=rb_d.ap())
            wv = pp.tile([128, 8, 256], F16, tag="wv")
            nc.sync.dma_start(out=wv[:], in_=wv_d.ap())
            bv = pp.tile([1, 256], BF, tag="bv")
            nc.sync.dma_start(out=bv[:], in_=bv_d.ap())
            nc.sync.dma_start(out=wq[:, 2:4, :, :], in_=wq_d.ap()[:, 2:4, :, :])
            nc.sync.dma_start(out=wk[:, 2:4, :, :], in_=wk_d.ap()[:, 2:4, :, :])
            mask01 = pp.tile([128, 128], BF, tag="mask01")
            nc.sync.dma_start(out=mask01[:], in_=mask01_d.ap())
            nlam = pp.tile([128, 4], F32, tag="nlam")
            nc.sync.dma_start(out=nlam[:], in_=nlam_d.ap())
            epsb = pp.tile([128, 1], F32, tag="epsb")
            nc.sync.dma_start(out=epsb[:], in_=epsb_d.ap())
            ones1 = pp.tile([1, 128], BF, tag="ones1")
            nc.sync.dma_start(out=ones1[:], in_=ones1_d.ap())
            wo = pp.tile([128, 2, 1024], BF, tag="wo")
            nc.sync.dma_start(out=wo[:], in_=wo_d.ap())

            qT = pp.tile([128, NH, S], BF, tag="qT")
            kT = pp.tile([128, NH, S], BF, tag="kT")
            v_sb = pp.tile([128, 16, NH, 64], BF, tag="v")
            onescol = pp.tile([128, 1], BF, tag="onescol")
            nc.vector.memset(onescol[:], 1.0)

            shuf = [i ^ 1 for i in range(32)]

            # ---------- background work units (aux psum bank users)
            TAGB = {"aux": 1, "sc": 2, "o": 2, "sums": 1}

            def emit_qk_proj(c, f, tag="aux", copy_eng=None):
                """Project one q/k feature tile (head f%4, k if f>=4) for
                chunk c, apply bias + rope, write qT/kT."""
                cs = c * CH
                h = f % 4
                wsb = wq if f < 4 else wk
                pj = ps.tile([128, 512], F32, tag=tag, bufs=TAGB[tag],
                             name=f"pj_{c}_{f}")
                for g in range(8):
                    nc.tensor.matmul(
                        pj[:], lhsT=wsb[:, h, g, :],
                        rhs=xdr[:, g, cs:cs + CH],
                        start=(g == 0), stop=(g == 7))
                qc = wk_pool.tile([128, CH], BF, tag="qc", name=f"qc_{c}_{f}")
                if copy_eng == "act":
                    nc.scalar.activation(qc[:], pj[:], AF.Identity,
                                         bias=bqk[:, f:f + 1])
                else:
                    # GPSIMD cannot read PSUM on hw; drain via DVE with the
                    # bias folded in
                    nc.vector.tensor_scalar(out=qc[:], in0=pj[:],
                                            scalar1=bqk[:, f:f + 1],
                                            scalar2=None, op0=ADD)
                qs = wk_pool.tile([128, CH], BF, tag="qs", name=f"qs_{c}_{f}")
                nc.vector.stream_shuffle(qs[:], qc[:], shuf)
                t1 = wk_pool.tile([128, CH], BF, tag="t1", name=f"t1_{c}_{f}")
                nc.vector.tensor_tensor(out=t1[:], in0=qc[:],
                                        in1=ra[:, cs:cs + CH], op=MUL)
                t2 = wk_pool.tile([128, CH], BF, tag="t2", name=f"t2_{c}_{f}")
                if copy_eng == "act":
                    nc.vector.tensor_tensor(out=t2[:], in0=qs[:],
                                            in1=rb[:, cs:cs + CH], op=MUL)
                else:
                    nc.gpsimd.tensor_tensor(out=t2[:], in0=qs[:],
                                            in1=rb[:, cs:cs + CH], op=MUL)
                dst = (qT if f < 4 else kT)[:, h, cs:cs + CH]
                nc.vector.tensor_tensor(out=dst, in0=t1[:], in1=t2[:], op=ADD)

            def emit_v_proj(m, tag="aux"):
                """Project v for key block m (seq-major)."""
                ms = m * 128
                pj = ps.tile([128, 512], F32, tag=tag, bufs=TAGB[tag],
                             name=f"pv_{m}")
                for g in range(8):
                    nc.tensor.matmul(
                        pj[:, 0:256],
                        lhsT=xdr[:, g, ms:ms + 128],
                        rhs=wv[:, g, :],
                        start=(g == 0), stop=False)
                nc.tensor.matmul(pj[:, 0:256], lhsT=ones1[:], rhs=bv[:],
                                 start=False, stop=True)
                nc.vector.tensor_copy(
                    v_sb[:, m, :, :],
                    pj[:, 0:256].rearrange("p (h d) -> p h d", h=4))

            def emit_outproj(c, t, n, ofT, tag="aux", copy_eng=None):
                """Half out-projection for qsub t of chunk c."""
                cs = c * CH
                yp = ps.tile([128, 512], F32, tag=tag, bufs=TAGB[tag],
                             name=f"yp_{c}_{t}_{n}")
                nc.tensor.matmul(yp[:], lhsT=ofT[:, 0, :],
                                 rhs=wo[:, 0, n * 512:n * 512 + 512],
                                 start=True, stop=False)
                nc.tensor.matmul(yp[:], lhsT=ofT[:, 1, :],
                                 rhs=wo[:, 1, n * 512:n * 512 + 512],
                                 start=False, stop=True)
                ysb = wk_pool.tile([128, 512], BF, tag="ysb",
                                   name=f"ysb_{c}_{t}_{n}")
                if copy_eng == "act":
                    nc.scalar.copy(ysb[:], yp[:])
                else:
                    nc.vector.tensor_copy(ysb[:], yp[:])
                nc.sync.dma_start(
                    out=y_d.ap()[cs + t * 128:cs + t * 128 + 128,
                                 n * 512:n * 512 + 512],
                    in_=ysb[:])

            # two background queues: aux-bank users (rate-limited to one per
            # two attention units so the single bank never head-blocks the
            # PE) and free units (no psum). Entries are labeled so forced
            # drains keep emission ahead of first use.
            bg_aux = []
            bg_free = []
            ctr = [0]

            def aux_push(deadline, kind, key, thunk):
                bg_aux.append((deadline, kind, key, thunk))
                bg_aux.sort(key=lambda e: e[0])

            def drain_one():
                ctr[0] += 1
                if bg_free:
                    bg_free.pop(0)()
                if bg_aux and (ctr[0] % 2 == 0 or len(bg_aux) > 6):
                    bg_aux.pop(0)[3]()

            def force_aux(pred):
                while True:
                    idx = next((i for i, e in enumerate(bg_aux) if pred(e)),
                               None)
                    if idx is None:
                        break
                    for _ in range(idx + 1):
                        bg_aux.pop(0)[3]()

            seq = [0, 1, 2, 3]

            # startup: chunk-0 pass-0 inputs + first v blocks, spread over
            # the idle psum banks; everything else queued
            eager = [("qk", 0), ("qk", 4), ("v", 0), ("v", 1), ("v", 2),
                     ("v", 3), ("qk", 1), ("qk", 5), ("qk", 2), ("qk", 6)]
            tagc = ["aux", "sc", "o", "sums", "sc", "o"]
            for i, (kind, idx) in enumerate(eager):
                tg = tagc[i % len(tagc)]
                if kind == "qk":
                    emit_qk_proj(seq[0], idx, tag=tg,
                                 copy_eng="act" if i % 2 else None)
                else:
                    emit_v_proj(idx, tag=tg)
            for f in (3, 7):
                aux_push(16, "qk", (0, f), lambda f=f: emit_qk_proj(0, f))
            for m in range(4, 16):
                aux_push((m // 4) * 100 + 4 + 2 * (m % 4), "v", m,
                         lambda m=m: emit_v_proj(m))

            for ci, c in enumerate(seq):
                cs = c * CH
                njb = 4 * c + 4
                if ci + 1 < NCH:
                    cn = seq[ci + 1]
                    for i, f in enumerate((0, 4, 1, 5)):
                        aux_push(cn * 100 - 20 + i, "qk", (cn, f),
                                 lambda c=cn, f=f: emit_qk_proj(c, f))
                    for i, f in enumerate((2, 6, 3, 7)):
                        aux_push(cn * 100 + 16 + i, "qk", (cn, f),
                                 lambda c=cn, f=f: emit_qk_proj(c, f))

                ofT = [wk_pool.tile([128, 2, 128], BF, tag="ofT", bufs=9,
                                    name=f"ofT_{c}_{t}") for t in range(4)]
                for hp in range(2):  # head pair pass
                    heads = (2 * hp, 2 * hp + 1)
                    force_aux(lambda e, c=c, heads=heads:
                              e[1] == "qk" and e[2][0] == c
                              and (e[2][1] % 4) in heads)
                    # psum accumulators: o_t[j] holds qsubs {2j, 2j+1}
                    o_t = [ps.tile([128, 2, 2, 2, 64], F32, tag="o",
                                   bufs=2, name=f"o_{c}_{hp}_{j}")
                           for j in range(2)]
                    sums = ps.tile([128, 4, 2, 2], F32, tag="sums",
                                   name=f"sums_{c}_{hp}")
                    # first/last matmul per psum bank carry start/stop
                    o_started = [False, False]
                    sums_started = [False]
                    o_last = {(min(4 * c + 2 * j + 1, njb - 1), 1, 2 * j + 1, 1):
                              j for j in range(2)}
                    sums_last = (njb - 1, 1, 3, 1)

                    def emit_av(jb, hh, t0, at):
                        force_aux(lambda e, jb=jb: e[1] == "v" and e[2] <= jb)
                        h = 2 * hp + hh
                        for t in range(t0, 4):
                            j, u = t // 2, t % 2
                            for comp in range(2):
                                a_sl = at[:, comp, t * 128:t * 128 + 128]
                                st = not o_started[j]
                                o_started[j] = True
                                nc.tensor.matmul(
                                    o_t[j][:, u, comp, hh, :], lhsT=a_sl,
                                    rhs=v_sb[:, jb, h, :], start=st,
                                    stop=(jb, hh, t, comp) in o_last)
                                st = not sums_started[0]
                                sums_started[0] = True
                                nc.tensor.matmul(
                                    sums[:, t, comp, hh:hh + 1],
                                    lhsT=a_sl, rhs=onescol[:], start=st,
                                    stop=(jb, hh, t, comp) == sums_last)

                    pend = []
                    for jb in range(njb):
                        i0 = max(0, (jb - 4 * c) * 128)
                        t0 = i0 // 128
                        for hh in range(2):
                            h = 2 * hp + hh
                            sc = ps.tile([128, 2, 512], F32, tag="sc",
                                         bufs=2, name=f"sc_{c}_{jb}_{h}")
                            diag = jb >= 4 * c
                            for comp in range(2):
                                nc.tensor.matmul(
                                    sc[:, comp, i0:512],
                                    lhsT=kT[64 * comp:64 * comp + 64, h,
                                            jb * 128:jb * 128 + 128],
                                    rhs=qT[64 * comp:64 * comp + 64, h,
                                           cs + i0:cs + CH],
                                    start=True, stop=True)
                            at = wk_pool.tile([128, 2, 512], BF, tag="at",
                                              bufs=4, name=f"at_{c}_{jb}_{h}")
                            nc.scalar.activation(at[:, :, i0:512],
                                                 sc[:, :, i0:512], AF.Exp,
                                                 scale=ESCALE)
                            if diag:
                                # zero the strict upper triangle (key>query)
                                # of the diagonal 128-block, on Pool
                                nc.gpsimd.tensor_tensor(
                                    out=at[:, :, i0:i0 + 128],
                                    in0=at[:, :, i0:i0 + 128],
                                    in1=bcast_mid(mask01[:, 0:128], 2),
                                    op=MUL)
                            # software pipeline: attn@v lags 2 units so the
                            # next unit's scores aren't blocked behind it
                            pend.append((jb, hh, t0, at))
                            if len(pend) > 2:
                                emit_av(*pend.pop(0))
                            drain_one()
                    for args in pend:
                        emit_av(*args)

                    # ---- normalize the pass's two heads, all 4 qsubs
                    msq = wk_pool.tile([128, 4, 2], F32, tag="msq", bufs=2,
                                       name=f"msq_{c}_{hp}")
                    ocs = []
                    for t in range(4):
                        j, u = t // 2, t % 2
                        r1 = wk_pool.tile([128, 2], F32, tag="r1",
                                          bufs=4, name=f"r1_{c}_{hp}_{t}")
                        nc.vector.reciprocal(r1[:], sums[:, t, 0, :])
                        r2r = wk_pool.tile([128, 2], F32, tag="r2r",
                                           bufs=4, name=f"r2r_{c}_{hp}_{t}")
                        nc.vector.reciprocal(r2r[:], sums[:, t, 1, :])
                        r2 = wk_pool.tile([128, 2], F32, tag="r2",
                                          bufs=4, name=f"r2_{c}_{hp}_{t}")
                        nc.vector.tensor_tensor(
                            out=r2[:], in0=nlam[:, 2 * hp:2 * hp + 2],
                            in1=r2r[:], op=MUL)
                        u1 = wk_pool.tile([128, 2, 64], BF, tag="u1",
                                          bufs=2, name=f"u1_{c}_{hp}_{t}")
                        nc.vector.tensor_tensor(
                            out=u1[:], in0=o_t[j][:, u, 0, :, :],
                            in1=bcast_free(r1[:, 0:2], 64), op=MUL)
                        u2 = wk_pool.tile([128, 2, 64], BF, tag="u2",
                                          bufs=2, name=f"u2_{c}_{hp}_{t}")
                        nc.vector.tensor_tensor(
                            out=u2[:], in0=o_t[j][:, u, 1, :, :],
                            in1=bcast_free(r2[:, 0:2], 64), op=MUL)
                        oc = wk_pool.tile([128, 2, 64], BF, tag="oc",
                                          bufs=10, name=f"oc_{c}_{hp}_{t}")
                        nc.vector.tensor_tensor(out=oc[:], in0=u1[:],
                                                in1=u2[:], op=ADD)
                        sq = wk_pool.tile([128, 2, 64], BF, tag="sq", bufs=2,
                                          name=f"sq_{c}_{hp}_{t}")
                        nc.vector.tensor_tensor(out=sq[:], in0=oc[:],
                                                in1=oc[:], op=MUL)
                        nc.vector.tensor_reduce(
                            out=msq[:, t, :], in_=sq[:],
                            axis=mybir.AxisListType.X, op=ADD)
                        ocs.append(oc)
                        drain_one()

                    # stage B (Act rsqrt + scale + transpose) deferred so it
                    # doesn't head-block ready exps in the Act queue
                    def stage_b(c=c, hp=hp, msq=msq, ocs=ocs, ofT=ofT):
                        lnt = wk_pool.tile([128, 4, 2], F32, tag="lnt",
                                           bufs=2, name=f"lnt_{c}_{hp}")
                        nc.scalar.activation(lnt[:], msq[:], AF.Ln,
                                             bias=epsb[:, 0:1],
                                             scale=1.0 / 64.0)
                        rmsi = wk_pool.tile([128, 4, 2], BF, tag="rmsi",
                                            bufs=2, name=f"rmsi_{c}_{hp}")
                        nc.scalar.activation(rmsi[:], lnt[:], AF.Exp,
                                             scale=-0.5)
                        for t in range(4):
                            ocn = wk_pool.tile([128, 2, 64], BF, tag="ocn",
                                               bufs=6, name=f"ocn_{c}_{hp}_{t}")
                            nc.vector.tensor_tensor(
                                out=ocn[:], in0=ocs[t][:],
                                in1=bcast_free(rmsi[:, t, 0:2], 64), op=MUL)
                            # DMA XBAR transpose [128q,128dv] -> [128dv,128q]
                            nc.sync.dma_start_transpose(
                                ofT[t][:, hp, :],
                                ocn.rearrange("p a b -> p (a b)"))
                    bg_free.insert(0, stage_b)

                if ci < NCH - 1:
                    for t in range(4):
                        for n in range(2):
                            aux_push(seq[ci + 1] * 100 + 50 + t * 2 + n,
                                     "op", c,
                                     lambda c=c, t=t, n=n, of=ofT[t]:
                                     emit_outproj(c, t, n, of))
                else:
                    while bg_free:
                        bg_free.pop(0)()
                    while bg_aux:
                        bg_aux.pop(0)[3]()
                    for t in range(4):
                        for n in range(2):
                            u = t * 2 + n
                            emit_outproj(c, t, n, ofT[t],
                                         tag=tagc[u % len(tagc)],
                                         copy_eng="act" if u % 2 else None)

            while bg_free:
                bg_free.pop(0)()
            while bg_aux:
                bg_aux.pop(0)[3]()

    nc.compile()
    return nc


def _prep_inputs(x, Wq, bq, Wk, bk, Wv, bv, Wo, bo, head_norm_w,
                 lq1, lk1, lq2, lk2):
    lam_full = (LAMBDA_INIT
                + np.exp(np.sum(lq1 * lk1, -1))
                - np.exp(np.sum(lq2 * lk2, -1)))  # [H]

    # feature permutation within each head's 128 q/k features: comp c's
    # rope pair (j, j+32) -> adjacent partitions (64c+2j, 64c+2j+1)
    perm = np.empty(128, np.int64)
    for p in range(128):
        comp, j, half = p // 64, (p % 64) // 2, p % 2
        perm[p] = comp * 64 + j + 32 * half

    half = 32
    inv_freq = 1.0 / (10000.0 ** (np.arange(half, dtype=np.float64) / half))
    ang = np.arange(S, dtype=np.float64)[None, :] * inv_freq[:, None]  # [32,S]
    ra = np.empty((128, S), np.float32)
    rbm = np.empty((128, S), np.float32)
    for p in range(128):
        j, e = (p % 64) // 2, p % 2
        ra[p] = np.cos(ang[j])
        rbm[p] = np.sin(ang[j]) * (1.0 if e else -1.0)

    mask01 = np.ones((128, 128), np.float32)
    for k in range(128):
        mask01[k, :k] = 0.0  # mask where key k > query q

    in_maps = []
    for core in range(NCORES):
        b = core // 4
        h0 = 4 * (core % 4)
        rq = slice(h0 * 128, h0 * 128 + 512)
        rv = slice(h0 * 64, h0 * 64 + 256)

        xb = x[b]  # [S, D]
        xdr = np.ascontiguousarray(
            xb.T.reshape(8, 128, S).transpose(1, 0, 2)).astype(F16)

        wq_l = Wq[rq] * WS  # [512, 1024]
        wk_l = Wk[rq] * WS
        # permuted feature rows per head
        wq_p = np.concatenate([wq_l[h * 128 + perm] for h in range(4)], 0)
        wk_p = np.concatenate([wk_l[h * 128 + perm] for h in range(4)], 0)
        # [128p, 4h, 8g, 128m]: [p,h,g,m] = W[h*128+m, g*128+p]
        wq_a = np.ascontiguousarray(
            wq_p.reshape(4, 128, 8, 128).transpose(3, 0, 2, 1)).astype(F16)
        wk_a = np.ascontiguousarray(
            wk_p.reshape(4, 128, 8, 128).transpose(3, 0, 2, 1)).astype(F16)

        wv_l = Wv[rv] * WS  # [256, 1024]
        wv_a = np.ascontiguousarray(
            wv_l.reshape(256, 8, 128).transpose(2, 1, 0)).astype(F16)

        hnw = head_norm_w[h0:h0 + 4].reshape(256)
        wo_l = Wo[:, rv].T * (hnw * (1.0 - LAMBDA_INIT))[:, None]  # [256,1024]
        wo_a = np.ascontiguousarray(
            wo_l.reshape(2, 128, 1024).transpose(1, 0, 2)).astype(BF16)

        bqk = np.empty((128, 8), np.float32)
        for f in range(8):
            src = bq if f < 4 else bk
            bqk[:, f] = src[rq][(f % 4) * 128 + perm] * WS

        nlam = np.tile((-lam_full[h0:h0 + 4]).astype(np.float32)[None, :],
                       (128, 1))

        in_maps.append({
            "xdr": xdr,
            "wq": wq_a,
            "wk": wk_a,
            "wv": wv_a,
            "wo": wo_a,
            "bqk": bqk,
            "bv": (bv[rv] * WS).reshape(1, 256).astype(BF16),
            "ra": ra.astype(BF16),
            "rb": rbm.astype(BF16),
            "mask01": mask01.astype(BF16),
            "nlam": nlam,
            "epsb": np.full((128, 1), EPS * WS * WS, np.float32),
            "ones1": np.ones((1, 128), BF16),
        })
    return in_maps


def kernel(**inputs):
    from concourse.bass_utils import run_bass_kernel_spmd

    if "nc" not in _cache:
        _cache["nc"] = _build()
    nc = _cache["nc"]

    inputs = {k: np.asarray(v) for k, v in inputs.items()}
    in_maps = _prep_inputs(**inputs)
    res = run_bass_kernel_spmd(nc, in_maps, list(range(NCORES)))

    bo = inputs["bo"]
    y = np.zeros((B, S, D), np.float32)
    for b in range(B):
        acc = np.zeros((S, D), np.float32)
        for c in range(4 * b, 4 * b + 4):
            acc += res.results[c]["y"].astype(np.float32)
        y[b] = acc + bo[None, :]
    return y


# revision 38
# speedup vs baseline: 1.9570x; 1.0209x over previous
"""Differential self-attention on 8 Trainium2 NeuronCores (v2).

Sharding: batch x head-group. Core c handles batch b = c//4 and heads
h0 = 4*(c%4) .. h0+4. Per core: q/k/v projections (fp8 DoubleRow
matmuls, weights pre-scaled x64 on host), RoPE (feature pairs
interleaved so rotate-half is a DVE stream_shuffle), causal scores per
128-key block (bf16, mask added as a -1e7 upper-triangular matmul into
the score psum), exp on the Act engine, position-major attn@v
(out = [queries, dims], so softmax denominators and RMS scales are
per-partition scalars), differential combine + RMS norm on DVE, PE
transpose, partial out-projection (head_norm and (1-lambda_init)
folded into Wo). Host sums the 4 partial y per batch and adds bo.
"""

import math

import numpy as np
import ml_dtypes

B, S, D = 2, 2048, 1024
H, HD = 16, 64
NCORES = 8
NH = 4            # heads per core
LAMBDA_INIT = 0.2
EPS = 1e-6
CH = 512          # query chunk
NCH = S // CH
QS = 128          # query sub-block
WS = 64.0         # fp8 weight pre-scale (keeps W out of e4m3 subnormals)
BF16 = ml_dtypes.bfloat16
E4M3 = ml_dtypes.float8_e4m3
MASKV = -1.0e7

_cache = {}


def _build():
    import contextlib

    import concourse.bass as bass
    import concourse.tile as tile
    from concourse import bacc, mybir

    F32 = mybir.dt.float32
    BF = mybir.dt.bfloat16
    FP8 = mybir.dt.float8e4
    AF = mybir.ActivationFunctionType
    DRM = mybir.MatmulPerfMode.DoubleRow
    MUL = mybir.AluOpType.mult
    ADD = mybir.AluOpType.add
    DIV = mybir.AluOpType.divide

    nc = bacc.Bacc("TRN2", debug=False, num_devices=NCORES)

    x8_d = nc.dram_tensor("x8", [128, 8, S], FP8, kind="ExternalInput")
    xr_d = nc.dram_tensor("xr", [128, 8, S], FP8, kind="ExternalInput")
    wq8_d = nc.dram_tensor("wq8", [128, 4, 4, 2, 128], FP8, kind="ExternalInput")
    wqr_d = nc.dram_tensor("wqr", [128, 4, 4, 2, 128], FP8, kind="ExternalInput")
    wk8_d = nc.dram_tensor("wk8", [128, 4, 4, 2, 128], FP8, kind="ExternalInput")
    wkr_d = nc.dram_tensor("wkr", [128, 4, 4, 2, 128], FP8, kind="ExternalInput")
    wv8_d = nc.dram_tensor("wv8", [128, 4, 2, 256], FP8, kind="ExternalInput")
    wvr_d = nc.dram_tensor("wvr", [128, 4, 2, 256], FP8, kind="ExternalInput")
    wo_d = nc.dram_tensor("wo", [128, 2, 1024], BF, kind="ExternalInput")
    bqk_d = nc.dram_tensor("bqk", [128, 8], F32, kind="ExternalInput")
    bv_d = nc.dram_tensor("bv", [1, 256], BF, kind="ExternalInput")
    ra_d = nc.dram_tensor("ra", [128, S], BF, kind="ExternalInput")
    rb_d = nc.dram_tensor("rb", [128, S], BF, kind="ExternalInput")
    mask01_d = nc.dram_tensor("mask01", [128, 128], BF, kind="ExternalInput")
    nlam_d = nc.dram_tensor("nlam", [128, 4], F32, kind="ExternalInput")
    epsb_d = nc.dram_tensor("epsb", [128, 1], F32, kind="ExternalInput")
    ones1_d = nc.dram_tensor("ones1", [1, 128], BF, kind="ExternalInput")
    y_d = nc.dram_tensor("y", [S, D], BF, kind="ExternalOutput")

    ESCALE = 0.125 / (WS * WS)  # 1/sqrt(hd) folded with fp8 weight scale

    def bcast_free(ap, n):
        """Broadcast a [...]-shaped AP along a new innermost free dim."""
        return bass.AP(tensor=ap.tensor, offset=ap.offset, ap=ap.ap + [[0, n]])

    def bcast_mid(ap, n):
        """Broadcast a [p, m] AP to [p, n, m] (stride-0 middle dim)."""
        return bass.AP(tensor=ap.tensor, offset=ap.offset,
                       ap=[ap.ap[0], [0, n]] + ap.ap[1:])

    with tile.TileContext(nc) as tc:
        ctx = contextlib.ExitStack()
        with ctx:
            pp = ctx.enter_context(tc.tile_pool(name="pers", bufs=1))
            wk_pool = ctx.enter_context(tc.tile_pool(name="wkp", bufs=2))
            ps = ctx.enter_context(tc.tile_pool(name="ps", bufs=1, space="PSUM"))

            # ---- persistent SBUF (DMA order tuned so chunk-0 pass-0
            # projections can start as early as possible)
            wq8 = pp.tile([128, 4, 4, 2, 128], FP8, tag="wq8")
            nc.sync.dma_start(out=wq8[:], in_=wq8_d.ap())
            wk8 = pp.tile([128, 4, 4, 2, 128], FP8, tag="wk8")
            nc.sync.dma_start(out=wk8[:], in_=wk8_d.ap())
            bqk = pp.tile([128, 8], F32, tag="bqk")
            nc.sync.dma_start(out=bqk[:], in_=bqk_d.ap())
            x8 = pp.tile([128, 8, S], FP8, tag="x8")
            for g in range(8):
                nc.sync.dma_start(out=x8[:, g, :], in_=x8_d.ap()[:, g, :])
            xr = pp.tile([128, 8, S], FP8, tag="xr")
            for g in range(8):
                nc.sync.dma_start(out=xr[:, g, :], in_=xr_d.ap()[:, g, :])
            wqr = pp.tile([128, 4, 4, 2, 128], FP8, tag="wqr")
            nc.sync.dma_start(out=wqr[:], in_=wqr_d.ap())
            wkr = pp.tile([128, 4, 4, 2, 128], FP8, tag="wkr")
            nc.sync.dma_start(out=wkr[:], in_=wkr_d.ap())
            ra = pp.tile([128, S], BF, tag="ra")
            nc.sync.dma_start(out=ra[:], in_=ra_d.ap())
            rb = pp.tile([128, S], BF, tag="rb")
            nc.sync.dma_start(out=rb[:], in_=rb_d.ap())
            wv8 = pp.tile([128, 4, 2, 256], FP8, tag="wv8")
            nc.sync.dma_start(out=wv8[:], in_=wv8_d.ap())
            wvr = pp.tile([128, 4, 2, 256], FP8, tag="wvr")
            nc.sync.dma_start(out=wvr[:], in_=wvr_d.ap())
            bv = pp.tile([1, 256], BF, tag="bv")
            nc.sync.dma_start(out=bv[:], in_=bv_d.ap())
            mask01 = pp.tile([128, 128], BF, tag="mask01")
            nc.sync.dma_start(out=mask01[:], in_=mask01_d.ap())
            nlam = pp.tile([128, 4], F32, tag="nlam")
            nc.sync.dma_start(out=nlam[:], in_=nlam_d.ap())
            epsb = pp.tile([128, 1], F32, tag="epsb")
            nc.sync.dma_start(out=epsb[:], in_=epsb_d.ap())
            ones1 = pp.tile([1, 128], BF, tag="ones1")
            nc.sync.dma_start(out=ones1[:], in_=ones1_d.ap())
            wo = pp.tile([128, 2, 1024], BF, tag="wo")
            nc.sync.dma_start(out=wo[:], in_=wo_d.ap())

            qT = pp.tile([128, NH, S], BF, tag="qT")
            kT = pp.tile([128, NH, S], BF, tag="kT")
            v_sb = pp.tile([128, 16, NH, 64], BF, tag="v")
            onescol = pp.tile([128, 1], BF, tag="onescol")
            nc.vector.memset(onescol[:], 1.0)

            shuf = [i ^ 1 for i in range(32)]

            # ---------- background work units (aux psum bank users)
            TAGB = {"aux": 1, "sc": 2, "o": 2, "sums": 1}

            def emit_qk_proj(c, f, tag="aux", copy_eng=None):
                """Project one q/k feature tile (head f%4, k if f>=4) for
                chunk c, apply bias + rope, write qT/kT."""
                cs = c * CH
                h = f % 4
                whi = wq8 if f < 4 else wk8
                wlo = wqr if f < 4 else wkr
                pj = ps.tile([128, 512], F32, tag=tag, bufs=TAGB[tag],
                             name=f"pj_{c}_{f}")
                # hi*hi + hi*lo + lo*hi fp8 DoubleRow passes (~f16 precision)
                for pi, (wsb, xsb) in enumerate(
                        ((wq8 if f < 4 else wk8, x8), (whi, xr), (wlo, x8))):
                    for g in range(4):
                        nc.tensor.matmul(
                            pj[:], lhsT=wsb[:, h, g, :, :],
                            rhs=xsb[:, 2 * g:2 * g + 2, cs:cs + CH],
                            start=(pi == 0 and g == 0),
                            stop=(pi == 2 and g == 3), perf_mode=DRM)
                qc = wk_pool.tile([128, CH], BF, tag="qc", name=f"qc_{c}_{f}")
                if copy_eng == "act":
                    nc.scalar.activation(qc[:], pj[:], AF.Identity,
                                         bias=bqk[:, f:f + 1])
                else:
                    # GPSIMD cannot read PSUM on hw; drain via DVE with the
                    # bias folded in
                    nc.vector.tensor_scalar(out=qc[:], in0=pj[:],
                                            scalar1=bqk[:, f:f + 1],
                                            scalar2=None, op0=ADD)
                qs = wk_pool.tile([128, CH], BF, tag="qs", name=f"qs_{c}_{f}")
                nc.vector.stream_shuffle(qs[:], qc[:], shuf)
                t1 = wk_pool.tile([128, CH], BF, tag="t1", name=f"t1_{c}_{f}")
                nc.vector.tensor_tensor(out=t1[:], in0=qc[:],
                                        in1=ra[:, cs:cs + CH], op=MUL)
                t2 = wk_pool.tile([128, CH], BF, tag="t2", name=f"t2_{c}_{f}")
                if copy_eng == "act":
                    nc.vector.tensor_tensor(out=t2[:], in0=qs[:],
                                            in1=rb[:, cs:cs + CH], op=MUL)
                else:
                    nc.gpsimd.tensor_tensor(out=t2[:], in0=qs[:],
                                            in1=rb[:, cs:cs + CH], op=MUL)
                dst = (qT if f < 4 else kT)[:, h, cs:cs + CH]
                nc.vector.tensor_tensor(out=dst, in0=t1[:], in1=t2[:], op=ADD)

            def emit_v_proj(m, tag="aux"):
                """Project v for key block m (seq-major)."""
                ms = m * 128
                pj = ps.tile([128, 512], F32, tag=tag, bufs=TAGB[tag],
                             name=f"pv_{m}")
                for pi, (xsb, wsb) in enumerate(
                        ((x8, wv8), (xr, wv8), (x8, wvr))):
                    for g in range(4):
                        nc.tensor.matmul(
                            pj[:, 0:256],
                            lhsT=xsb[:, 2 * g:2 * g + 2, ms:ms + 128],
                            rhs=wsb[:, g, :, :],
                            start=(pi == 0 and g == 0), stop=False,
                            perf_mode=DRM)
                nc.tensor.matmul(pj[:, 0:256], lhsT=ones1[:], rhs=bv[:],
                                 start=False, stop=True)
                nc.vector.tensor_copy(
                    v_sb[:, m, :, :],
                    pj[:, 0:256].rearrange("p (h d) -> p h d", h=4))

            def emit_outproj(c, t, n, ofT, tag="aux", copy_eng=None):
                """Half out-projection for qsub t of chunk c."""
                cs = c * CH
                yp = ps.tile([128, 512], F32, tag=tag, bufs=TAGB[tag],
                             name=f"yp_{c}_{t}_{n}")
                nc.tensor.matmul(yp[:], lhsT=ofT[:, 0, :],
                                 rhs=wo[:, 0, n * 512:n * 512 + 512],
                                 start=True, stop=False)
                nc.tensor.matmul(yp[:], lhsT=ofT[:, 1, :],
                                 rhs=wo[:, 1, n * 512:n * 512 + 512],
                                 start=False, stop=True)
                ysb = wk_pool.tile([128, 512], BF, tag="ysb",
                                   name=f"ysb_{c}_{t}_{n}")
                if copy_eng == "act":
                    nc.scalar.copy(ysb[:], yp[:])
                else:
                    nc.vector.tensor_copy(ysb[:], yp[:])
                nc.sync.dma_start(
                    out=y_d.ap()[cs + t * 128:cs + t * 128 + 128,
                                 n * 512:n * 512 + 512],
                    in_=ysb[:])

            # two background queues: aux-bank users (rate-limited to one per
            # two attention units so the single bank never head-blocks the
            # PE) and free units (no psum). Entries are labeled so forced
            # drains keep emission ahead of first use.
            bg_aux = []
            bg_free = []
            ctr = [0]

            def aux_push(deadline, kind, key, thunk):
                bg_aux.append((deadline, kind, key, thunk))
                bg_aux.sort(key=lambda e: e[0])

            def drain_one():
                ctr[0] += 1
                if bg_free:
                    bg_free.pop(0)()
                if bg_aux and (ctr[0] % 2 == 0 or len(bg_aux) > 6):
                    bg_aux.pop(0)[3]()

            def force_aux(pred):
                while True:
                    idx = next((i for i, e in enumerate(bg_aux) if pred(e)),
                               None)
                    if idx is None:
                        break
                    for _ in range(idx + 1):
                        bg_aux.pop(0)[3]()

            seq = [0, 1, 2, 3]

            # startup: chunk-0 pass-0 inputs + first v blocks, spread over
            # the idle psum banks; everything else queued
            eager = [("qk", 0), ("qk", 4), ("v", 0), ("v", 1), ("v", 2),
                     ("v", 3), ("qk", 1), ("qk", 5), ("qk", 2), ("qk", 6)]
            tagc = ["aux", "sc", "o", "sums", "sc", "o"]
            for i, (kind, idx) in enumerate(eager):
                tg = tagc[i % len(tagc)]
                if kind == "qk":
                    emit_qk_proj(seq[0], idx, tag=tg,
                                 copy_eng="act" if i % 2 else None)
                else:
                    emit_v_proj(idx, tag=tg)
            for f in (3, 7):
                aux_push(16, "qk", (0, f), lambda f=f: emit_qk_proj(0, f))
            for m in range(4, 16):
                aux_push((m // 4) * 100 + 4 + 2 * (m % 4), "v", m,
                         lambda m=m: emit_v_proj(m))

            for ci, c in enumerate(seq):
                cs = c * CH
                njb = 4 * c + 4
                if ci + 1 < NCH:
                    cn = seq[ci + 1]
                    for i, f in enumerate((0, 4, 1, 5)):
                        aux_push(cn * 100 - 20 + i, "qk", (cn, f),
                                 lambda c=cn, f=f: emit_qk_proj(c, f))
                    for i, f in enumerate((2, 6, 3, 7)):
                        aux_push(cn * 100 + 16 + i, "qk", (cn, f),
                                 lambda c=cn, f=f: emit_qk_proj(c, f))

                ofT = [wk_pool.tile([128, 2, 128], BF, tag="ofT", bufs=9,
                                    name=f"ofT_{c}_{t}") for t in range(4)]
                for hp in range(2):  # head pair pass
                    heads = (2 * hp, 2 * hp + 1)
                    force_aux(lambda e, c=c, heads=heads:
                              e[1] == "qk" and e[2][0] == c
                              and (e[2][1] % 4) in heads)
                    # psum accumulators: o_t[j] holds qsubs {2j, 2j+1}
                    o_t = [ps.tile([128, 2, 2, 2, 64], F32, tag="o",
                                   bufs=2, name=f"o_{c}_{hp}_{j}")
                           for j in range(2)]
                    sums = ps.tile([128, 4, 2, 2], F32, tag="sums",
                                   name=f"sums_{c}_{hp}")
                    # first/last matmul per psum bank carry start/stop
                    o_started = [False, False]
                    sums_started = [False]
                    o_last = {(min(4 * c + 2 * j + 1, njb - 1), 1, 2 * j + 1, 1):
                              j for j in range(2)}
                    sums_last = (njb - 1, 1, 3, 1)

                    def emit_av(jb, hh, t0, at):
                        force_aux(lambda e, jb=jb: e[1] == "v" and e[2] <= jb)
                        h = 2 * hp + hh
                        for t in range(t0, 4):
                            j, u = t // 2, t % 2
                            for comp in range(2):
                                a_sl = at[:, comp, t * 128:t * 128 + 128]
                                st = not o_started[j]
                                o_started[j] = True
                                nc.tensor.matmul(
                                    o_t[j][:, u, comp, hh, :], lhsT=a_sl,
                                    rhs=v_sb[:, jb, h, :], start=st,
                                    stop=(jb, hh, t, comp) in o_last)
                                st = not sums_started[0]
                                sums_started[0] = True
                                nc.tensor.matmul(
                                    sums[:, t, comp, hh:hh + 1],
                                    lhsT=a_sl, rhs=onescol[:], start=st,
                                    stop=(jb, hh, t, comp) == sums_last)

                    pend = []
                    for jb in range(njb):
                        i0 = max(0, (jb - 4 * c) * 128)
                        t0 = i0 // 128
                        for hh in range(2):
                            h = 2 * hp + hh
                            sc = ps.tile([128, 2, 512], F32, tag="sc",
                                         bufs=2, name=f"sc_{c}_{jb}_{h}")
                            diag = jb >= 4 * c
                            for comp in range(2):
                                nc.tensor.matmul(
                                    sc[:, comp, i0:512],
                                    lhsT=kT[64 * comp:64 * comp + 64, h,
                                            jb * 128:jb * 128 + 128],
                                    rhs=qT[64 * comp:64 * comp + 64, h,
                                           cs + i0:cs + CH],
                                    start=True, stop=True)
                            at = wk_pool.tile([128, 2, 512], BF, tag="at",
                                              bufs=4, name=f"at_{c}_{jb}_{h}")
                            nc.scalar.activation(at[:, :, i0:512],
                                                 sc[:, :, i0:512], AF.Exp,
                                                 scale=ESCALE)
                            if diag:
                                # zero the strict upper triangle (key>query)
                                # of the diagonal 128-block, on Pool
                                nc.gpsimd.tensor_tensor(
                                    out=at[:, :, i0:i0 + 128],
                                    in0=at[:, :, i0:i0 + 128],
                                    in1=bcast_mid(mask01[:, 0:128], 2),
                                    op=MUL)
                            # software pipeline: attn@v lags 2 units so the
                            # next unit's scores aren't blocked behind it
                            pend.append((jb, hh, t0, at))
                            if len(pend) > 2:
                                emit_av(*pend.pop(0))
                            drain_one()
                    for args in pend:
                        emit_av(*args)

                    # ---- normalize the pass's two heads, all 4 qsubs
                    msq = wk_pool.tile([128, 4, 2], F32, tag="msq", bufs=2,
                                       name=f"msq_{c}_{hp}")
                    ocs = []
                    for t in range(4):
                        j, u = t // 2, t % 2
                        r1 = wk_pool.tile([128, 2], F32, tag="r1",
                                          bufs=4, name=f"r1_{c}_{hp}_{t}")
                        nc.vector.reciprocal(r1[:], sums[:, t, 0, :])
                        r2r = wk_pool.tile([128, 2], F32, tag="r2r",
                                           bufs=4, name=f"r2r_{c}_{hp}_{t}")
                        nc.vector.reciprocal(r2r[:], sums[:, t, 1, :])
                        r2 = wk_pool.tile([128, 2], F32, tag="r2",
                                          bufs=4, name=f"r2_{c}_{hp}_{t}")
                        nc.vector.tensor_tensor(
                            out=r2[:], in0=nlam[:, 2 * hp:2 * hp + 2],
                            in1=r2r[:], op=MUL)
                        u1 = wk_pool.tile([128, 2, 64], BF, tag="u1",
                                          bufs=2, name=f"u1_{c}_{hp}_{t}")
                        nc.vector.tensor_tensor(
                            out=u1[:], in0=o_t[j][:, u, 0, :, :],
                            in1=bcast_free(r1[:, 0:2], 64), op=MUL)
                        u2 = wk_pool.tile([128, 2, 64], BF, tag="u2",
                                          bufs=2, name=f"u2_{c}_{hp}_{t}")
                        nc.vector.tensor_tensor(
                            out=u2[:], in0=o_t[j][:, u, 1, :, :],
                            in1=bcast_free(r2[:, 0:2], 64), op=MUL)
                        oc = wk_pool.tile([128, 2, 64], BF, tag="oc",
                                          bufs=10, name=f"oc_{c}_{hp}_{t}")
                        nc.vector.tensor_tensor(out=oc[:], in0=u1[:],
                                                in1=u2[:], op=ADD)
                        sq = wk_pool.tile([128, 2, 64], BF, tag="sq", bufs=2,
                                          name=f"sq_{c}_{hp}_{t}")
                        nc.vector.tensor_tensor(out=sq[:], in0=oc[:],
                                                in1=oc[:], op=MUL)
                        nc.vector.tensor_reduce(
                            out=msq[:, t, :], in_=sq[:],
                            axis=mybir.AxisListType.X, op=ADD)
                        ocs.append(oc)
                        drain_one()

                    # stage B (Act rsqrt + scale + transpose) deferred so it
                    # doesn't head-block ready exps in the Act queue
                    def stage_b(c=c, hp=hp, msq=msq, ocs=ocs, ofT=ofT):
                        lnt = wk_pool.tile([128, 4, 2], F32, tag="lnt",
                                           bufs=2, name=f"lnt_{c}_{hp}")
                        nc.scalar.activation(lnt[:], msq[:], AF.Ln,
                                             bias=epsb[:, 0:1],
                                             scale=1.0 / 64.0)
                        rmsi = wk_pool.tile([128, 4, 2], BF, tag="rmsi",
                                            bufs=2, name=f"rmsi_{c}_{hp}")
                        nc.scalar.activation(rmsi[:], lnt[:], AF.Exp,
                                             scale=-0.5)
                        for t in range(4):
                            ocn = wk_pool.tile([128, 2, 64], BF, tag="ocn",
                                               bufs=6, name=f"ocn_{c}_{hp}_{t}")
                            nc.vector.tensor_tensor(
                                out=ocn[:], in0=ocs[t][:],
                                in1=bcast_free(rmsi[:, t, 0:2], 64), op=MUL)
                            # DMA XBAR transpose [128q,128dv] -> [128dv,128q]
                            nc.sync.dma_start_transpose(
                                ofT[t][:, hp, :],
                                ocn.rearrange("p a b -> p (a b)"))
                    bg_free.insert(0, stage_b)

                if ci < NCH - 1:
                    for t in range(4):
                        for n in range(2):
                            aux_push(seq[ci + 1] * 100 + 50 + t * 2 + n,
                                     "op", c,
                                     lambda c=c, t=t, n=n, of=ofT[t]:
                                     emit_outproj(c, t, n, of))
                else:
                    while bg_free:
                        bg_free.pop(0)()
                    while bg_aux:
                        bg_aux.pop(0)[3]()
                    for t in range(4):
                        for n in range(2):
                            u = t * 2 + n
                            emit_outproj(c, t, n, ofT[t],
                                         tag=tagc[u % len(tagc)],
                                         copy_eng="act" if u % 2 else None)

            while bg_free:
                bg_free.pop(0)()
            while bg_aux:
                bg_aux.pop(0)[3]()

    nc.compile()
    return nc


def _prep_inputs(x, Wq, bq, Wk, bk, Wv, bv, Wo, bo, head_norm_w,
                 lq1, lk1, lq2, lk2):
    lam_full = (LAMBDA_INIT
                + np.exp(np.sum(lq1 * lk1, -1))
                - np.exp(np.sum(lq2 * lk2, -1)))  # [H]

    # feature permutation within each head's 128 q/k features: comp c's
    # rope pair (j, j+32) -> adjacent partitions (64c+2j, 64c+2j+1)
    perm = np.empty(128, np.int64)
    for p in range(128):
        comp, j, half = p // 64, (p % 64) // 2, p % 2
        perm[p] = comp * 64 + j + 32 * half

    half = 32
    inv_freq = 1.0 / (10000.0 ** (np.arange(half, dtype=np.float64) / half))
    ang = np.arange(S, dtype=np.float64)[None, :] * inv_freq[:, None]  # [32,S]
    ra = np.empty((128, S), np.float32)
    rbm = np.empty((128, S), np.float32)
    for p in range(128):
        j, e = (p % 64) // 2, p % 2
        ra[p] = np.cos(ang[j])
        rbm[p] = np.sin(ang[j]) * (1.0 if e else -1.0)

    mask01 = np.ones((128, 128), np.float32)
    for k in range(128):
        mask01[k, :k] = 0.0  # mask where key k > query q

    in_maps = []
    for core in range(NCORES):
        b = core // 4
        h0 = 4 * (core % 4)
        rq = slice(h0 * 128, h0 * 128 + 512)
        rv = slice(h0 * 64, h0 * 64 + 256)

        xb = x[b]  # [S, D]
        xdr = np.ascontiguousarray(
            xb.T.reshape(8, 128, S).transpose(1, 0, 2)).astype(np.float32)
        x8 = xdr.astype(E4M3)
        xr = (xdr - x8.astype(np.float32)).astype(E4M3)

        wq_l = Wq[rq] * WS  # [512, 1024]
        wk_l = Wk[rq] * WS
        # permuted feature rows per head
        wq_p = np.concatenate([wq_l[h * 128 + perm] for h in range(4)], 0)
        wk_p = np.concatenate([wk_l[h * 128 + perm] for h in range(4)], 0)
        # [128p, 4h, 4g, 2i, 128m]: [p,h,g,i,m] = W[h*128+m, (2g+i)*128+p]
        wq_a = np.ascontiguousarray(
            wq_p.reshape(4, 128, 8, 128).transpose(3, 0, 2, 1)
            .reshape(128, 4, 4, 2, 128)).astype(np.float32)
        wk_a = np.ascontiguousarray(
            wk_p.reshape(4, 128, 8, 128).transpose(3, 0, 2, 1)
            .reshape(128, 4, 4, 2, 128)).astype(np.float32)
        wq8 = wq_a.astype(E4M3)
        wq_r = (wq_a - wq8.astype(np.float32)).astype(E4M3)
        wk8 = wk_a.astype(E4M3)
        wk_r = (wk_a - wk8.astype(np.float32)).astype(E4M3)

        wv_l = Wv[rv] * WS  # [256, 1024]
        wv_a = np.ascontiguousarray(
            wv_l.reshape(256, 8, 128).transpose(2, 1, 0)
            .reshape(128, 4, 2, 256)).astype(np.float32)
        wv8 = wv_a.astype(E4M3)
        wv_r = (wv_a - wv8.astype(np.float32)).astype(E4M3)

        hnw = head_norm_w[h0:h0 + 4].reshape(256)
        wo_l = Wo[:, rv].T * (hnw * (1.0 - LAMBDA_INIT))[:, None]  # [256,1024]
        wo_a = np.ascontiguousarray(
            wo_l.reshape(2, 128, 1024).transpose(1, 0, 2)).astype(BF16)

        bqk = np.empty((128, 8), np.float32)
        for f in range(8):
            src = bq if f < 4 else bk
            bqk[:, f] = src[rq][(f % 4) * 128 + perm] * WS

        nlam = np.tile((-lam_full[h0:h0 + 4]).astype(np.float32)[None, :],
                       (128, 1))

        in_maps.append({
            "x8": x8,
            "xr": xr,
            "wq8": wq8,
            "wqr": wq_r,
            "wk8": wk8,
            "wkr": wk_r,
            "wv8": wv8,
            "wvr": wv_r,
            "wo": wo_a,
            "bqk": bqk,
            "bv": (bv[rv] * WS).reshape(1, 256).astype(BF16),
            "ra": ra.astype(BF16),
            "rb": rbm.astype(BF16),
            "mask01": mask01.astype(BF16),
            "nlam": nlam,
            "epsb": np.full((128, 1), EPS * WS * WS, np.float32),
            "ones1": np.ones((1, 128), BF16),
        })
    return in_maps


def kernel(**inputs):
    from concourse.bass_utils import run_bass_kernel_spmd

    if "nc" not in _cache:
        _cache["nc"] = _build()
    nc = _cache["nc"]

    inputs = {k: np.asarray(v) for k, v in inputs.items()}
    in_maps = _prep_inputs(**inputs)
    res = run_bass_kernel_spmd(nc, in_maps, list(range(NCORES)))

    bo = inputs["bo"]
    y = np.zeros((B, S, D), np.float32)
    for b in range(B):
        acc = np.zeros((S, D), np.float32)
        for c in range(4 * b, 4 * b + 4):
            acc += res.results[c]["y"].astype(np.float32)
        y[b] = acc + bo[None, :]
    return y
